# revision 1
# baseline (speedup 1.0000x reference)
"""Trainium2 Bass kernel for nn_CustomRNN_88871463289370.

Reference computation (B=1024, T=256, H=512, HORIZON=24):
    h_0 = 0
    h_{t+1} = tanh(outer(x[:, t], Wx_w) + h_t @ Wh_w.T + (Wx_b + Wh_b))
    out = h_T @ fc_w.T + fc_b                      # [B, 24]

Strategy (data-parallel over batch, 8 cores x 128 rows each):
  * Feature-major ("transposed") on-chip layout: hT[k] tiles are
    [128 hidden-features (partition), 128 batch (free)], k = 0..3.
  * Per step, per output chunk m (4 chunks of 128 hidden units):
      psum[m] = WxB_chunk.T @ [x_t ; ones]        (K=2 matmul: x-outer + bias)
              + sum_k WhT[k, m-chunk].T @ hT[k]   (4 K=128 matmuls, PSUM accum)
    then hT'[m] = tanh(psum[m]) on ScalarE.
  * Two merged [128, 256] Tanh activations per step (chunks m01 / m23) so
    the ACT engine work (~1.0us/step) hides under PE work (~1.5us/step).
  * Matmuls are emitted "k-phase-major" (all k in {0,1} for every m, then
    all k in {2,3}) so the first half of the next step only depends on the
    first ACT of this step -- shortens the serial chain.
  * All x rows live SBUF-resident for the whole kernel as a [2, T, B]
    tile ([x_t ; ones] rows), loaded by one DMA up front -- no per-step
    DMAs and no DMA waits on the hot path.
  * Final projection: 4 K=128 matmuls into a [24, 128] PSUM tile plus a
    per-partition-bias Identity activation.

All host-side reshaping/transposition/casting happens in kernel() below;
the device kernel sees pre-massaged tensors.
"""

import numpy as np
import ml_dtypes

HIDDEN = 512
HORIZON = 24
B_FULL = 1024
T_FULL = 256
N_CORES = 8
B_CORE = B_FULL // N_CORES  # 128
KC = HIDDEN // 128          # 4 chunks of the hidden dim

_COMPILED = {}


def build_kernel(T=T_FULL, use_bf16=True, reps=1, b_use=B_CORE,
                 pack_phase0=False):
    """Build the Bass module. `reps` repeats the whole time loop
    (python-unrolled; each rep recomputes h from scratch) and `b_use`
    narrows the streamed batch columns -- both benchmarking-only knobs.
    `pack_phase0` places the four K=2 x-outer matmuls in distinct 32-row
    PE groups (tile_position) so they run concurrently in the array --
    DISABLED by default: two consecutive NRT_EXEC_UNIT_UNRECOVERABLE device
    wedges followed packed executions (the concurrent same-bank start=True
    pending-zero clear is the suspect; the doc's packing recipe uses one
    PSUM bank per row-tile), while the unpacked kernel is stable."""
    import concourse.bass as bass
    import concourse.mybir as mybir
    import concourse.tile as tile
    from concourse.bass import ts

    dt = mybir.dt.bfloat16 if use_bf16 else mybir.dt.float32
    f32 = mybir.dt.float32

    nc = bass.Bass("TRN2", target_bir_lowering=False, debug=False,
                   num_devices=N_CORES)

    NP = 128 if pack_phase0 else 2   # partition extent of x2/wxb tiles

    # ---- DRAM I/O (per-core shapes; host pre-massages layouts) ----
    # x2T[0, t] = x[:, t], x2T[1, t] = ones; shape [2, T, B_CORE]
    x2T_d = nc.dram_tensor("x2T", [2, T, B_CORE], dt, kind="ExternalInput").ap()
    # WxB[0] = Wx_w, WxB[1] = Wx_b + Wh_b, shape [2, H]
    wxb_d = nc.dram_tensor("wxb", [2, HIDDEN], dt, kind="ExternalInput").ap()
    # WhT arranged [128, KC, H]: whT[p, k, m] = Wh_w[m, k*128+p]
    whT_d = nc.dram_tensor("whT", [128, KC, HIDDEN], dt, kind="ExternalInput").ap()
    # fcT arranged [128, KC, HORIZON]: fcT[p, k, n] = fc_w[n, k*128+p]
    fcT_d = nc.dram_tensor("fcT", [128, KC, HORIZON], dt, kind="ExternalInput").ap()
    # fc_b as column [HORIZON, 1] fp32
    fcb_d = nc.dram_tensor("fcb", [HORIZON, 1], f32, kind="ExternalInput").ap()
    # output [HORIZON, b_use] fp32 (host transposes/concats)
    out_d = nc.dram_tensor("out", [HORIZON, b_use], f32, kind="ExternalOutput").ap()

    with tile.TileContext(nc) as tc:
        with (
            tc.tile_pool(name="consts", bufs=1) as cpool,
            tc.tile_pool(name="h", bufs=3) as hpool,
            tc.tile_pool(name="ps", bufs=3, space="PSUM") as pspool,
            tc.tile_pool(name="fin", bufs=1) as finpool,
        ):
            # ---- load constants into SBUF ----
            # all x rows resident on partitions 0-1; when packing phase 0,
            # replicate [x; ones] and [Wx; bias] to partitions 32/64/96 so
            # each K=2 matmul can address its own 32-row PE group.
            x2_sb = cpool.tile([NP, T, B_CORE], dt)
            nc.sync.dma_start(x2_sb[0:2], x2T_d[:])
            wxb_sb = cpool.tile([NP, HIDDEN], dt)
            nc.sync.dma_start(wxb_sb[0:2], wxb_d[:])
            if pack_phase0:
                for q in (32, 64, 96):
                    nc.sync.dma_start(x2_sb[q:q + 2], x2_sb[0:2])
                    nc.sync.dma_start(wxb_sb[q:q + 2], wxb_sb[0:2])
            whT_sb = cpool.tile([128, KC, HIDDEN], dt)
            nc.sync.dma_start(whT_sb[:], whT_d[:])
            fcT_sb = cpool.tile([128, KC, HORIZON], dt)
            nc.sync.dma_start(fcT_sb[:], fcT_d[:])
            fcb_sb = cpool.tile([HORIZON, 1], f32)
            nc.sync.dma_start(fcb_sb[:], fcb_d[:])
            # Touch fcb on ScalarE right away so the DMA wait lands here,
            # not on the final bias activation (which already carries a PE
            # wait; the AC instruction struct fits only one sync wait).
            fcb_scratch = cpool.tile([1, 1], f32)
            nc.scalar.activation(fcb_scratch[:], fcb_sb[0:1, 0:1],
                                 mybir.ActivationFunctionType.Identity)

            h01 = None  # [128, 256] tiles: hT chunks 0|1 and 2|3
            h23 = None

            for _rep in range(reps):
                h01, h23 = _emit_steps(nc, mybir, ts, T, dt, f32, x2_sb,
                                       wxb_sb, whT_sb, hpool, pspool, b_use,
                                       pack_phase0)

            # ---- final projection: out[n, b] = sum_k fcT[k].T @ hT[k] + b ----
            ps_fc = pspool.tile([HORIZON, b_use], f32, tag="psA")
            hs = (h01, h01, h23, h23)
            for k in range(KC):
                nc.tensor.matmul(ps_fc[:], fcT_sb[:, k, :],
                                 hs[k][:, ts(k % 2, b_use)],
                                 start=(k == 0), stop=(k == KC - 1))
            out_sb = finpool.tile([HORIZON, b_use], f32)
            nc.scalar.activation(out_sb[:], ps_fc[:],
                                 mybir.ActivationFunctionType.Identity,
                                 bias=fcb_sb[:])
            nc.sync.dma_start(out_d[:], out_sb[:])

    _strip_redundant_self_waits(nc)
    return nc


def _emit_steps(nc, mybir, ts, T, dt, f32, x2_sb, wxb_sb, whT_sb,
                hpool, pspool, b_use=B_CORE, pack_phase0=False):
    """Emit the T sequential RNN steps; returns the final (h01, h23) tiles."""
    h01 = h23 = None
    for t in range(T):

        psA = pspool.tile([128, 2 * b_use], f32, tag="psA")
        psB = pspool.tile([128, 2 * b_use], f32, tag="psB")
        ps_of = lambda m: (psA, psB)[m // 2][:, ts(m % 2, b_use)]

        # start/stop are per PSUM *bank*: exactly one start=True on the
        # first matmul into each tile and one stop=True on the last
        # (start marks the whole 2KB bank pending-zero).
        # phase 0: x-outer + bias (K=2); when packed, the four matmuls
        # sit in distinct 32-row PE groups and run concurrently.
        for m in range(4):
            q = 32 * m if pack_phase0 else 0
            nc.tensor.matmul(ps_of(m), wxb_sb[q:q + 2, ts(m, 128)],
                             x2_sb[q:q + 2, t, :b_use],
                             start=(m % 2 == 0),
                             stop=(t == 0 and m % 2 == 1),
                             tile_position=(q, 0) if pack_phase0 else None)
        if t > 0:
            # phase 1: k in {0, 1} -> depends on h01 of prev step
            for m in range(4):
                for k in (0, 1):
                    nc.tensor.matmul(ps_of(m),
                                     whT_sb[:, k, ts(m, 128)],
                                     h01[:, ts(k, b_use)],
                                     start=False, stop=False)
            # phase 2: k in {2, 3} -> depends on h23 of prev step
            for m in range(4):
                for k in (2, 3):
                    nc.tensor.matmul(ps_of(m),
                                     whT_sb[:, k, ts(m, 128)],
                                     h23[:, ts(k - 2, b_use)],
                                     start=False,
                                     stop=(k == 3 and m % 2 == 1))
                if m == 1:
                    h01_new = hpool.tile([128, 2 * b_use], dt, tag="h01")
                    nc.scalar.activation(h01_new[:], psA[:],
                                         mybir.ActivationFunctionType.Tanh)
        else:
            h01_new = hpool.tile([128, 2 * b_use], dt, tag="h01")
            nc.scalar.activation(h01_new[:], psA[:],
                                 mybir.ActivationFunctionType.Tanh)
        h23_new = hpool.tile([128, 2 * b_use], dt, tag="h23")
        nc.scalar.activation(h23_new[:], psB[:],
                             mybir.ActivationFunctionType.Tanh)
        h01, h23 = h01_new, h23_new
    return h01, h23


_SELF_SEM_PREFIX = {
    "InstActivation": "Activation",
    "InstMatmult": "PE",
    "InstLdweights": "PE",
    "InstTensorTensor": "DVE",
    "InstTensorScalarPtr": "DVE",
    "InstTensorCopy": "DVE",
}


def _strip_redundant_self_waits(nc):
    """Drop same-engine semaphore waits from instructions that carry more
    than one sync wait.

    Rationale: the HW engine instruction structs (MM/AC) hold only ONE
    sync-wait command; walrus refuses to codegen instructions with two.
    Tile emits a wait on the instruction's own engine sem for WAW/WAR on
    recycled tile-pool slots, but each engine executes its queue strictly
    in order, so ordering vs. its own earlier instructions is guaranteed
    without the wait.  Cross-engine waits are preserved; sem update counts
    are untouched (no other wait thresholds shift).
    """
    # Semaphore updated by the final DMA store of the "out" tensor; the
    # kernel-tail drain only genuinely needs this one (everything else is
    # transitively ordered: input DMAs -> compute -> final ACT -> out DMA).
    out_dma_sems = set()
    for b in nc.m.functions[0].blocks:
        for i in b.instructions:
            if type(i).__name__ != "InstDMACopy":
                continue
            names = [getattr(ap, "memref", "") for ap in i.outs]
            if "out" in names:
                si = i.sync_info
                if si:
                    out_dma_sems.update(u.ant_name for u in si.on_update)

    for b in nc.m.functions[0].blocks:
        for i in b.instructions:
            si = i.sync_info
            if si is None:
                continue
            ow = si.on_wait
            if len(ow) < 2:
                continue
            tname = type(i).__name__
            if tname == "InstDrain" and any(
                w.ant_name in out_dma_sems for w in ow
            ):
                si.on_wait = [w for w in ow if w.ant_name in out_dma_sems][:1]
                continue
            if tname == "InstDMACopy":
                # Keep the compute-engine wait (real data dependency);
                # drop stale cross-queue DMAHW waits (no data dependency:
                # all earlier DMAs here are input preloads this store
                # does not read, and same-ring descriptors are ordered
                # by the ring itself).
                kept = [w for w in ow if not w.ant_name.startswith("DMA")]
                if kept and len(kept) < len(ow):
                    si.on_wait = kept
                continue
            self_prefix = _SELF_SEM_PREFIX.get(tname)
            if self_prefix is None:
                continue
            kept = [w for w in ow if not w.ant_name.startswith(self_prefix)]
            if kept and len(kept) < len(ow):
                si.on_wait = kept


def _prep_inputs(x, Wx_w, Wx_b, Wh_w, Wh_b, fc_w, fc_b, T, use_bf16):
    """Host-side shard + layout massaging. Returns per-core input maps."""
    dt = ml_dtypes.bfloat16 if use_bf16 else np.float32
    bias = (Wx_b + Wh_b).astype(np.float32)

    wxb = np.stack([Wx_w.astype(np.float32), bias]).astype(dt)          # [2, H]
    whT = (Wh_w.T.astype(np.float32)
           .reshape(KC, 128, HIDDEN).transpose(1, 0, 2).copy().astype(dt))
    fcT = (fc_w.T.astype(np.float32)
           .reshape(KC, 128, HORIZON).transpose(1, 0, 2).copy().astype(dt))
    fcb = fc_b.astype(np.float32).reshape(HORIZON, 1).copy()

    in_maps = []
    for c in range(N_CORES):
        xs = x[c * B_CORE:(c + 1) * B_CORE, :T]                          # [128, T]
        x2T = np.empty((2, T, B_CORE), dtype=np.float32)
        x2T[0] = xs.T
        x2T[1] = 1.0
        in_maps.append({
            "x2T": x2T.astype(dt),
            "wxb": wxb,
            "whT": whT,
            "fcT": fcT,
            "fcb": fcb,
        })
    return in_maps


def kernel(x, Wx_w, Wx_b, Wh_w, Wh_b, fc_w, fc_b, _T=T_FULL, _bf16=True,
           _trace=False):
    from concourse.bass_utils import run_bass_kernel_spmd

    key = (_T, _bf16)
    if key not in _COMPILED:
        _COMPILED[key] = build_kernel(T=_T, use_bf16=_bf16)
    nc = _COMPILED[key]

    in_maps = _prep_inputs(x, Wx_w, Wx_b, Wh_w, Wh_b, fc_w, fc_b, _T, _bf16)
    res = run_bass_kernel_spmd(nc, in_maps, list(range(N_CORES)), trace=_trace)
    outs = [res.results[c]["out"] for c in range(N_CORES)]               # [24, 128] each
    full = np.concatenate(outs, axis=1).T.astype(np.float32).copy()      # [1024, 24]
    kernel._last_result = res
    return full



# revision 2
# speedup vs baseline: 1.3143x; 1.3143x over previous
"""Trainium2 Bass kernel for nn_CustomRNN_88871463289370.

Reference computation (B=1024, T=256, H=512, HORIZON=24):
    h_0 = 0
    h_{t+1} = tanh(outer(x[:, t], Wx_w) + h_t @ Wh_w.T + (Wx_b + Wh_b))
    out = h_T @ fc_w.T + fc_b                      # [B, 24]

Strategy (data-parallel over batch, 8 cores x 128 rows each; inside each
core the 128 batch rows are further split into G=3 independent recurrence
groups of 43/43/42 columns):
  * Feature-major on-chip layout per group g: h_g is [128 hidden-feature
    partitions, 4 k-chunks x n_g batch cols]; full hidden state of one
    group fits a single PSUM bank [128, 4*n_g] fp32.
  * Per step, per group: 4 K=2 matmuls (x-outer + fused bias via an
    appended ones-row) + 16 K=128 matmuls (4 output chunks x 4 k-chunks)
    accumulate into the group's bank, then ONE [128, 4*n_g] Tanh on the
    ACT engine produces h_g for the next step.
  * Why groups: the per-step serial chain (last matmul -> PSUM drain ->
    tanh -> SBUF drain -> first matmul of next step) is ~980 ns for a
    43-col group, while the PE has ~1066 ns of matmul work per step.
    With 3 phase-shifted groups the PE always has another group's
    matmuls to run while one group's tanh round-trips, so the tensor
    engine never idles (the 2-group baseline idled ~400 ns/step).
  * All x rows live SBUF-resident for the whole kernel as a [2, T, B]
    tile ([x_t ; ones] rows), loaded by one DMA up front -- no per-step
    DMAs on the hot path.
  * Final projection: per group, 4 K=128 matmuls into a shared
    [24, 128] PSUM tile plus a per-partition-bias Identity activation.

All host-side reshaping/transposition/casting happens in kernel() below;
the device kernel sees pre-massaged tensors.
"""

import numpy as np
import ml_dtypes

HIDDEN = 512
HORIZON = 24
B_FULL = 1024
T_FULL = 256
N_CORES = 8
B_CORE = B_FULL // N_CORES  # 128
KC = HIDDEN // 128          # 4 chunks of the hidden dim
GROUPS = (43, 43, 42)       # batch-column split inside each core

_COMPILED = {}


def build_kernel(T=T_FULL, use_bf16=True):
    import concourse.bass as bass
    import concourse.mybir as mybir
    import concourse.tile as tile
    from concourse.bass import ts

    dt = mybir.dt.bfloat16 if use_bf16 else mybir.dt.float32
    f32 = mybir.dt.float32

    nc = bass.Bass("TRN2", target_bir_lowering=False, debug=False,
                   num_devices=N_CORES)

    # ---- DRAM I/O (per-core shapes; host pre-massages layouts) ----
    # x2T[0, t] = x[:, t], x2T[1, t] = ones; shape [2, T, B_CORE]
    x2T_d = nc.dram_tensor("x2T", [2, T, B_CORE], dt, kind="ExternalInput").ap()
    # WxB[0] = Wx_w, WxB[1] = Wx_b + Wh_b, shape [2, H]
    wxb_d = nc.dram_tensor("wxb", [2, HIDDEN], dt, kind="ExternalInput").ap()
    # WhT arranged [128, KC, H]: whT[p, k, m] = Wh_w[m, k*128+p]
    whT_d = nc.dram_tensor("whT", [128, KC, HIDDEN], dt, kind="ExternalInput").ap()
    # fcT arranged [128, KC, HORIZON]: fcT[p, k, n] = fc_w[n, k*128+p]
    fcT_d = nc.dram_tensor("fcT", [128, KC, HORIZON], dt, kind="ExternalInput").ap()
    # fc_b as column [HORIZON, 1] fp32
    fcb_d = nc.dram_tensor("fcb", [HORIZON, 1], f32, kind="ExternalInput").ap()
    # output [HORIZON, B_CORE] fp32 (host transposes/concats)
    out_d = nc.dram_tensor("out", [HORIZON, B_CORE], f32, kind="ExternalOutput").ap()

    ng = len(GROUPS)
    goff = [sum(GROUPS[:i]) for i in range(ng)]  # column offsets per group

    with tile.TileContext(nc) as tc:
        with (
            tc.tile_pool(name="consts", bufs=1) as cpool,
            tc.tile_pool(name="h", bufs=3) as hpool,
            tc.tile_pool(name="ps", bufs=2, space="PSUM") as pspool,
            tc.tile_pool(name="fin", bufs=1) as finpool,
        ):
            # ---- load constants into SBUF ----
            x2_sb = cpool.tile([2, T, B_CORE], dt)
            nc.sync.dma_start(x2_sb[:], x2T_d[:])
            wxb_sb = cpool.tile([2, HIDDEN], dt)
            nc.sync.dma_start(wxb_sb[:], wxb_d[:])
            whT_sb = cpool.tile([128, KC, HIDDEN], dt)
            nc.sync.dma_start(whT_sb[:], whT_d[:])
            fcT_sb = cpool.tile([128, KC, HORIZON], dt)
            nc.sync.dma_start(fcT_sb[:], fcT_d[:])
            fcb_sb = cpool.tile([HORIZON, 1], f32)
            nc.sync.dma_start(fcb_sb[:], fcb_d[:])
            # Touch fcb on ScalarE right away so the DMA wait lands here,
            # not on the final bias activation (which already carries a PE
            # wait; the AC instruction struct fits only one sync wait).
            fcb_scratch = cpool.tile([1, 1], f32)
            nc.scalar.activation(fcb_scratch[:], fcb_sb[0:1, 0:1],
                                 mybir.ActivationFunctionType.Identity)

            # h[g] tiles: [128, KC, n_g] (hidden-feature partition,
            # k-chunk x batch-col free)
            h = [None] * ng

            for t in range(T):
                for g in range(ng):
                    n = GROUPS[g]
                    o = goff[g]
                    ps = pspool.tile([128, KC * n], f32, tag=f"ps{g}")

                    # phase 0: x-outer + bias (K=2); first matmul into the
                    # bank carries start=True (marks whole bank pending-zero)
                    for m in range(KC):
                        nc.tensor.matmul(ps[:, ts(m, n)],
                                         wxb_sb[0:2, ts(m, 128)],
                                         x2_sb[0:2, t, o:o + n],
                                         start=(m == 0),
                                         stop=(t == 0 and m == KC - 1))
                    if t > 0:
                        for m in range(KC):
                            for k in range(KC):
                                nc.tensor.matmul(ps[:, ts(m, n)],
                                                 whT_sb[:, k, ts(m, 128)],
                                                 h[g][:, k, :],
                                                 start=False,
                                                 stop=(m == KC - 1 and
                                                       k == KC - 1))
                    h_new = hpool.tile([128, KC, n], dt, tag=f"h{g}")
                    nc.scalar.activation(h_new[:], ps[:],
                                         mybir.ActivationFunctionType.Tanh)
                    h[g] = h_new

            # ---- final projection: out[n, b] = sum_k fcT[k].T @ h[k] + b ----
            ps_fc = pspool.tile([HORIZON, B_CORE], f32, tag="psfc")
            for g in range(ng):
                n = GROUPS[g]
                o = goff[g]
                for k in range(KC):
                    nc.tensor.matmul(ps_fc[:, o:o + n],
                                     fcT_sb[:, k, :],
                                     h[g][:, k, :],
                                     start=(g == 0 and k == 0),
                                     stop=(g == ng - 1 and k == KC - 1))
            out_sb = finpool.tile([HORIZON, B_CORE], f32)
            nc.scalar.activation(out_sb[:], ps_fc[:],
                                 mybir.ActivationFunctionType.Identity,
                                 bias=fcb_sb[:])
            nc.sync.dma_start(out_d[:], out_sb[:])

    _strip_redundant_self_waits(nc)
    return nc


_SELF_SEM_PREFIX = {
    "InstActivation": "Activation",
    "InstMatmult": "PE",
    "InstLdweights": "PE",
    "InstTensorTensor": "DVE",
    "InstTensorScalarPtr": "DVE",
    "InstTensorCopy": "DVE",
}


def _strip_redundant_self_waits(nc):
    """Drop same-engine semaphore waits from instructions that carry more
    than one sync wait.

    Rationale: the HW engine instruction structs (MM/AC) hold only ONE
    sync-wait command; walrus refuses to codegen instructions with two.
    Tile emits a wait on the instruction's own engine sem for WAW/WAR on
    recycled tile-pool slots, but each engine executes its queue strictly
    in order, so ordering vs. its own earlier instructions is guaranteed
    without the wait.  Cross-engine waits are preserved; sem update counts
    are untouched (no other wait thresholds shift).
    """
    # Semaphore updated by the final DMA store of the "out" tensor; the
    # kernel-tail drain only genuinely needs this one (everything else is
    # transitively ordered: input DMAs -> compute -> final ACT -> out DMA).
    out_dma_sems = set()
    for b in nc.m.functions[0].blocks:
        for i in b.instructions:
            if type(i).__name__ != "InstDMACopy":
                continue
            names = [getattr(ap, "memref", "") for ap in i.outs]
            if "out" in names:
                si = i.sync_info
                if si:
                    out_dma_sems.update(u.ant_name for u in si.on_update)

    for b in nc.m.functions[0].blocks:
        for i in b.instructions:
            si = i.sync_info
            if si is None:
                continue
            ow = si.on_wait
            if len(ow) < 2:
                continue
            tname = type(i).__name__
            if tname == "InstDrain" and any(
                w.ant_name in out_dma_sems for w in ow
            ):
                si.on_wait = [w for w in ow if w.ant_name in out_dma_sems][:1]
                continue
            if tname == "InstDMACopy":
                # Keep the compute-engine wait (real data dependency);
                # drop stale cross-queue DMAHW waits (no data dependency:
                # all earlier DMAs here are input preloads this store
                # does not read, and same-ring descriptors are ordered
                # by the ring itself).
                kept = [w for w in ow if not w.ant_name.startswith("DMA")]
                if kept and len(kept) < len(ow):
                    si.on_wait = kept
                continue
            self_prefix = _SELF_SEM_PREFIX.get(tname)
            if self_prefix is None:
                continue
            kept = [w for w in ow if not w.ant_name.startswith(self_prefix)]
            if kept and len(kept) < len(ow):
                si.on_wait = kept


def _prep_inputs(x, Wx_w, Wx_b, Wh_w, Wh_b, fc_w, fc_b, T, use_bf16):
    """Host-side shard + layout massaging. Returns per-core input maps."""
    dt = ml_dtypes.bfloat16 if use_bf16 else np.float32
    bias = (Wx_b + Wh_b).astype(np.float32)

    wxb = np.stack([Wx_w.astype(np.float32), bias]).astype(dt)          # [2, H]
    whT = (Wh_w.T.astype(np.float32)
           .reshape(KC, 128, HIDDEN).transpose(1, 0, 2).copy().astype(dt))
    fcT = (fc_w.T.astype(np.float32)
           .reshape(KC, 128, HORIZON).transpose(1, 0, 2).copy().astype(dt))
    fcb = fc_b.astype(np.float32).reshape(HORIZON, 1).copy()

    in_maps = []
    for c in range(N_CORES):
        xs = x[c * B_CORE:(c + 1) * B_CORE, :T]                          # [128, T]
        x2T = np.empty((2, T, B_CORE), dtype=np.float32)
        x2T[0] = xs.T
        x2T[1] = 1.0
        in_maps.append({
            "x2T": x2T.astype(dt),
            "wxb": wxb,
            "whT": whT,
            "fcT": fcT,
            "fcb": fcb,
        })
    return in_maps


def kernel(x, Wx_w, Wx_b, Wh_w, Wh_b, fc_w, fc_b, _T=T_FULL, _bf16=True,
           _trace=False):
    from concourse.bass_utils import run_bass_kernel_spmd

    key = (_T, _bf16)
    if key not in _COMPILED:
        _COMPILED[key] = build_kernel(T=_T, use_bf16=_bf16)
    nc = _COMPILED[key]

    in_maps = _prep_inputs(x, Wx_w, Wx_b, Wh_w, Wh_b, fc_w, fc_b, _T, _bf16)
    res = run_bass_kernel_spmd(nc, in_maps, list(range(N_CORES)), trace=_trace)
    outs = [res.results[c]["out"] for c in range(N_CORES)]               # [24, 128] each
    full = np.concatenate(outs, axis=1).T.astype(np.float32).copy()      # [1024, 24]
    kernel._last_result = res
    return full


# revision 10
# speedup vs baseline: 1.4400x; 1.0956x over previous
"""Trainium2 Bass kernel for nn_CustomRNN_88871463289370.

Reference computation (B=1024, T=256, H=512, HORIZON=24):
    h_0 = 0
    h_{t+1} = tanh(outer(x[:, t], Wx_w) + h_t @ Wh_w.T + (Wx_b + Wh_b))
    out = h_T @ fc_w.T + fc_b                      # [B, 24]

Strategy (data-parallel over batch, 8 cores x 128 rows each; inside each
core the 128 batch rows are further split into G=3 independent recurrence
groups of 43/43/42 columns):
  * Feature-major on-chip layout per group g: h_g is [128 hidden-feature
    partitions, 4 k-chunks x n_g batch cols]; full hidden state of one
    group fits a single PSUM bank [128, 4*n_g] fp32.
  * Per step, per group: 4 K=2 bf16 matmuls (x-outer + fused bias via an
    appended ones-row) + 16 fp8e4m3 DoubleRow matmuls (4 output chunks x
    2 double-k-passes x {hi, lo} weight halves) accumulate into the
    group's bank, then ONE [128, 4*n_g] Tanh on the ACT engine produces
    h_g (written directly as fp8) for the next step.
  * fp8 numerics: Wh is split as W_hi = fp8(Wh), W_lo = fp8(Wh - W_hi)
    so the weights carry ~bf16 precision across the two accumulation
    passes; only h itself is quantized to e4m3.  DoubleRow runs at 0.5
    PE-cycles per output column, so the hi+lo scheme costs the same PE
    time as one bf16 pass would, and the measured end-to-end rel err is
    1.2e-2 (gate 2e-2).  The LAST step's tanh is written as bf16 and
    feeds the fc projection -- an fp8 final h alone would add ~2.1e-2
    error at the output (measured), so this bf16 tail is required.
  * Why groups: the per-step serial chain (last matmul -> PSUM drain ->
    tanh -> SBUF drain -> first matmul of next step) is ~840 ns for a
    43-col group, while the ACT engine has ~984 ns of tanh work per step
    (the bottleneck engine; PE has ~645 ns).  With 3 phase-shifted
    groups the ACT engine always has another group's tanh to run while
    one group's matmuls are in flight, so it never idles (the 2-group
    baseline serialized the chain and idled the PE ~400 ns/step).
  * All x rows live SBUF-resident for the whole kernel as a [2, T, B]
    tile ([x_t ; ones] rows), loaded by one DMA up front -- no per-step
    DMAs on the hot path.
  * Final projection: per group, 4 K=128 matmuls into a shared
    [24, 128] PSUM tile plus a per-partition-bias Identity activation.

All host-side reshaping/transposition/casting happens in kernel() below;
the device kernel sees pre-massaged tensors.
"""

import numpy as np
import ml_dtypes

HIDDEN = 512
HORIZON = 24
B_FULL = 1024
T_FULL = 256
N_CORES = 8
B_CORE = B_FULL // N_CORES  # 128
KC = HIDDEN // 128          # 4 chunks of the hidden dim
# batch-column split inside each core; widths must be EVEN: fp8 DoubleRow
# matmuls wedge the exec unit (NRT_EXEC_UNIT_UNRECOVERABLE) with an odd
# moving-operand width (empirically: n=43 wedges, 32/42/44 are fine)
GROUPS = (44, 42, 42)

_COMPILED = {}


def build_kernel(T=T_FULL, use_bf16=True):
    import concourse.bass as bass
    import concourse.mybir as mybir
    import concourse.tile as tile
    from concourse.bass import ts

    dt = mybir.dt.bfloat16 if use_bf16 else mybir.dt.float32
    f32 = mybir.dt.float32
    f8 = mybir.dt.float8e4
    DR = mybir.MatmulPerfMode.DoubleRow

    nc = bass.Bass("TRN2", target_bir_lowering=False, debug=False,
                   num_devices=N_CORES)

    # ---- DRAM I/O (per-core shapes; host pre-massages layouts) ----
    # x2T[0, t] = x[:, t], x2T[1, t] = ones; shape [2, T, B_CORE]
    x2T_d = nc.dram_tensor("x2T", [2, T, B_CORE], dt, kind="ExternalInput").ap()
    # WxB[0] = Wx_w, WxB[1] = Wx_b + Wh_b, shape [2, H]
    wxb_d = nc.dram_tensor("wxb", [2, HIDDEN], dt, kind="ExternalInput").ap()
    # Wh in fp8 DoubleRow layout [128, 2, 2, H]:
    # wh8*[p, q, i, m] = W*[m, (2q+i)*128 + p]  (hi/lo halves)
    wh8hi_d = nc.dram_tensor("wh8hi", [128, 2, 2, HIDDEN], f8,
                             kind="ExternalInput").ap()
    wh8lo_d = nc.dram_tensor("wh8lo", [128, 2, 2, HIDDEN], f8,
                             kind="ExternalInput").ap()
    # fcT arranged [128, KC, HORIZON]: fcT[p, k, n] = fc_w[n, k*128+p]
    fcT_d = nc.dram_tensor("fcT", [128, KC, HORIZON], dt, kind="ExternalInput").ap()
    # fc_b as column [HORIZON, 1] fp32
    fcb_d = nc.dram_tensor("fcb", [HORIZON, 1], f32, kind="ExternalInput").ap()
    # output [HORIZON, B_CORE] fp32 (host transposes/concats)
    out_d = nc.dram_tensor("out", [HORIZON, B_CORE], f32, kind="ExternalOutput").ap()

    ng = len(GROUPS)
    goff = [sum(GROUPS[:i]) for i in range(ng)]  # column offsets per group

    with tile.TileContext(nc) as tc:
        with (
            tc.tile_pool(name="consts", bufs=1) as cpool,
            tc.tile_pool(name="h", bufs=3) as hpool,
            tc.tile_pool(name="ps", bufs=2, space="PSUM") as pspool,
            tc.tile_pool(name="fin", bufs=1) as finpool,
        ):
            # ---- load constants into SBUF ----
            x2_sb = cpool.tile([2, T, B_CORE], dt)
            nc.sync.dma_start(x2_sb[:], x2T_d[:])
            wxb_sb = cpool.tile([2, HIDDEN], dt)
            nc.sync.dma_start(wxb_sb[:], wxb_d[:])
            wh8hi_sb = cpool.tile([128, 2, 2, HIDDEN], f8)
            nc.sync.dma_start(wh8hi_sb[:], wh8hi_d[:])
            wh8lo_sb = cpool.tile([128, 2, 2, HIDDEN], f8)
            nc.sync.dma_start(wh8lo_sb[:], wh8lo_d[:])
            fcT_sb = cpool.tile([128, KC, HORIZON], dt)
            nc.sync.dma_start(fcT_sb[:], fcT_d[:])
            fcb_sb = cpool.tile([HORIZON, 1], f32)
            nc.sync.dma_start(fcb_sb[:], fcb_d[:])
            # Touch fcb on ScalarE right away so the DMA wait lands here,
            # not on the final bias activation (which already carries a PE
            # wait; the AC instruction struct fits only one sync wait).
            fcb_scratch = cpool.tile([1, 1], f32)
            nc.scalar.activation(fcb_scratch[:], fcb_sb[0:1, 0:1],
                                 mybir.ActivationFunctionType.Identity)

            # h[g] tiles: [128, KC, n_g] (hidden-feature partition,
            # k-chunk x batch-col free)
            h = [None] * ng

            for t in range(T):
                for g in range(ng):
                    n = GROUPS[g]
                    o = goff[g]
                    ps = pspool.tile([128, KC * n], f32, tag=f"ps{g}")

                    # phase 0: x-outer + bias (K=2); first matmul into the
                    # bank carries start=True (marks whole bank pending-zero)
                    for m in range(KC):
                        nc.tensor.matmul(ps[:, ts(m, n)],
                                         wxb_sb[0:2, ts(m, 128)],
                                         x2_sb[0:2, t, o:o + n],
                                         start=(m == 0),
                                         stop=(t == 0 and m == KC - 1))
                    if t > 0:
                        for q in range(2):
                            for w8 in (wh8hi_sb, wh8lo_sb):
                                for m in range(KC):
                                    last = (q == 1 and w8 is wh8lo_sb and
                                            m == KC - 1)
                                    nc.tensor.matmul(
                                        ps[:, ts(m, n)],
                                        w8[:, q, :, ts(m, 128)],
                                        h[g][:, 2 * q:2 * q + 2, :],
                                        start=False, stop=last,
                                        perf_mode=DR)
                    if t < T - 1:
                        h_new = hpool.tile([128, KC, n], f8, tag=f"h{g}")
                    else:
                        # final h in bf16: feeds the fc projection, where
                        # an fp8 h would add ~2e-2 output error
                        h_new = hpool.tile([128, KC, n], dt, tag=f"hf{g}")
                    nc.scalar.activation(h_new[:], ps[:],
                                         mybir.ActivationFunctionType.Tanh)
                    h[g] = h_new

            # ---- final projection: out[n, b] = sum_k fcT[k].T @ h[k] + b ----
            ps_fc = pspool.tile([HORIZON, B_CORE], f32, tag="psfc")
            for g in range(ng):
                n = GROUPS[g]
                o = goff[g]
                for k in range(KC):
                    nc.tensor.matmul(ps_fc[:, o:o + n],
                                     fcT_sb[:, k, :],
                                     h[g][:, k, :],
                                     start=(g == 0 and k == 0),
                                     stop=(g == ng - 1 and k == KC - 1))
            out_sb = finpool.tile([HORIZON, B_CORE], f32)
            nc.scalar.activation(out_sb[:], ps_fc[:],
                                 mybir.ActivationFunctionType.Identity,
                                 bias=fcb_sb[:])
            nc.sync.dma_start(out_d[:], out_sb[:])

    _strip_redundant_self_waits(nc)
    return nc


_SELF_SEM_PREFIX = {
    "InstActivation": "Activation",
    "InstMatmult": "PE",
    "InstLdweights": "PE",
    "InstTensorTensor": "DVE",
    "InstTensorScalarPtr": "DVE",
    "InstTensorCopy": "DVE",
}


def _strip_redundant_self_waits(nc):
    """Drop same-engine semaphore waits from instructions that carry more
    than one sync wait.

    Rationale: the HW engine instruction structs (MM/AC) hold only ONE
    sync-wait command; walrus refuses to codegen instructions with two.
    Tile emits a wait on the instruction's own engine sem for WAW/WAR on
    recycled tile-pool slots, but each engine executes its queue strictly
    in order, so ordering vs. its own earlier instructions is guaranteed
    without the wait.  Cross-engine waits are preserved; sem update counts
    are untouched (no other wait thresholds shift).
    """
    # Semaphore updated by the final DMA store of the "out" tensor; the
    # kernel-tail drain only genuinely needs this one (everything else is
    # transitively ordered: input DMAs -> compute -> final ACT -> out DMA).
    out_dma_sems = set()
    for b in nc.m.functions[0].blocks:
        for i in b.instructions:
            if type(i).__name__ != "InstDMACopy":
                continue
            names = [getattr(ap, "memref", "") for ap in i.outs]
            if "out" in names:
                si = i.sync_info
                if si:
                    out_dma_sems.update(u.ant_name for u in si.on_update)

    for b in nc.m.functions[0].blocks:
        for i in b.instructions:
            si = i.sync_info
            if si is None:
                continue
            ow = si.on_wait
            if len(ow) < 2:
                continue
            tname = type(i).__name__
            if tname == "InstDrain" and any(
                w.ant_name in out_dma_sems for w in ow
            ):
                si.on_wait = [w for w in ow if w.ant_name in out_dma_sems][:1]
                continue
            if tname == "InstDMACopy":
                # Keep the compute-engine wait (real data dependency);
                # drop stale cross-queue DMAHW waits (no data dependency:
                # all earlier DMAs here are input preloads this store
                # does not read, and same-ring descriptors are ordered
                # by the ring itself).
                kept = [w for w in ow if not w.ant_name.startswith("DMA")]
                if kept and len(kept) < len(ow):
                    si.on_wait = kept
                continue
            self_prefix = _SELF_SEM_PREFIX.get(tname)
            if self_prefix is None:
                continue
            kept = [w for w in ow if not w.ant_name.startswith(self_prefix)]
            if kept and len(kept) < len(ow):
                si.on_wait = kept


def _prep_inputs(x, Wx_w, Wx_b, Wh_w, Wh_b, fc_w, fc_b, T, use_bf16):
    """Host-side shard + layout massaging. Returns per-core input maps."""
    dt = ml_dtypes.bfloat16 if use_bf16 else np.float32
    f8 = ml_dtypes.float8_e4m3
    bias = (Wx_b + Wh_b).astype(np.float32)

    wxb = np.stack([Wx_w.astype(np.float32), bias]).astype(dt)          # [2, H]

    # Wh hi/lo fp8 split in DoubleRow layout [p, q, i, m]:
    # wh8*[p, q, i, m] = W*[m, (2q+i)*128 + p]
    W = Wh_w.astype(np.float32)
    W_hi = W.astype(f8).astype(np.float32)
    W_lo = W - W_hi

    def dr_layout(Wf):                                  # [512, 512] -> [128,2,2,512]
        # WT[k, m] = Wf[m, k]; reshape k = (q, i, p)
        return (Wf.T.reshape(2, 2, 128, HIDDEN)
                .transpose(2, 0, 1, 3).copy().astype(f8))

    wh8hi = dr_layout(W_hi)
    wh8lo = dr_layout(W_lo)
    fcT = (fc_w.T.astype(np.float32)
           .reshape(KC, 128, HORIZON).transpose(1, 0, 2).copy().astype(dt))
    fcb = fc_b.astype(np.float32).reshape(HORIZON, 1).copy()

    in_maps = []
    for c in range(N_CORES):
        xs = x[c * B_CORE:(c + 1) * B_CORE, :T]                          # [128, T]
        x2T = np.empty((2, T, B_CORE), dtype=np.float32)
        x2T[0] = xs.T
        x2T[1] = 1.0
        in_maps.append({
            "x2T": x2T.astype(dt),
            "wxb": wxb,
            "wh8hi": wh8hi,
            "wh8lo": wh8lo,
            "fcT": fcT,
            "fcb": fcb,
        })
    return in_maps


def kernel(x, Wx_w, Wx_b, Wh_w, Wh_b, fc_w, fc_b, _T=T_FULL, _bf16=True,
           _trace=False):
    from concourse.bass_utils import run_bass_kernel_spmd

    key = (_T, _bf16)
    if key not in _COMPILED:
        _COMPILED[key] = build_kernel(T=_T, use_bf16=_bf16)
    nc = _COMPILED[key]

    in_maps = _prep_inputs(x, Wx_w, Wx_b, Wh_w, Wh_b, fc_w, fc_b, _T, _bf16)
    res = run_bass_kernel_spmd(nc, in_maps, list(range(N_CORES)), trace=_trace)
    outs = [res.results[c]["out"] for c in range(N_CORES)]               # [24, 128] each
    full = np.concatenate(outs, axis=1).T.astype(np.float32).copy()      # [1024, 24]
    kernel._last_result = res
    return full


# revision 26
# speedup vs baseline: 1.5121x; 1.0501x over previous
"""Trainium2 Bass kernel for nn_CustomRNN_88871463289370.

Reference computation (B=1024, T=256, H=512, HORIZON=24):
    h_0 = 0
    h_{t+1} = tanh(outer(x[:, t], Wx_w) + h_t @ Wh_w.T + (Wx_b + Wh_b))
    out = h_T @ fc_w.T + fc_b                      # [B, 24]

Strategy (data-parallel over batch, 8 cores x 128 rows each; inside each
core the 128 batch rows are further split into G=3 independent recurrence
groups of 46/40/42 columns):
  * Feature-major on-chip layout per group g: h_g is [128 hidden-feature
    partitions, 4 k-chunks x n_g batch cols]; full hidden state of one
    group fits a single PSUM bank [128, 4*n_g] fp32.
  * Per step, per group: 4 K=2 bf16 matmuls (x-outer + fused bias via an
    appended ones-row) + 16 fp8e4m3 DoubleRow matmuls (4 output chunks x
    2 double-k-passes x {hi, lo} weight halves) accumulate into the
    group's bank, then ONE [128, 4*n_g] Tanh on the ACT engine produces
    h_g (written directly as fp8) for the next step.
  * fp8 numerics: Wh is split as W_hi = fp8(Wh), W_lo = fp8(Wh - W_hi)
    so the weights carry ~bf16 precision across the two accumulation
    passes; only h itself is quantized to e4m3.  DoubleRow runs at 0.5
    PE-cycles per output column, so the hi+lo scheme costs the same PE
    time as one bf16 pass would, and the measured end-to-end rel err is
    1.2e-2 (gate 2e-2).  The LAST step's tanh is written as bf16 and
    feeds the fc projection -- an fp8 final h alone would add ~2.1e-2
    error at the output (measured), so this bf16 tail is required.
  * Why groups: the per-step serial chain (last matmul -> PSUM drain ->
    tanh -> SBUF drain -> first matmul of next step) is ~840 ns for a
    43-col group, while the ACT engine has ~984 ns of tanh work per step
    (the bottleneck engine; PE has ~645 ns).  With 3 phase-shifted
    groups the ACT engine always has another group's tanh to run while
    one group's matmuls are in flight, so it never idles (the 2-group
    baseline serialized the chain and idled the PE ~400 ns/step).
  * All x rows live SBUF-resident for the whole kernel as a [2, 4+T, B]
    tile that also packs the [Wx_w ; bias] chunks as 4 leading "steps",
    loaded in a head DMA (so step 0 is gated by one small transfer) plus
    an overlapped tail DMA -- no per-step DMAs on the hot path.
  * Final projection: per group, 4 K=128 bf16 matmuls emitted as soon as
    that group's final h lands, into two PSUM banks (g0+g1 | g2) so the
    first piece is copied out and stored while group 2 finishes; fc_b is
    added on the host after the gather.

All host-side reshaping/transposition/casting happens in kernel() below;
the device kernel sees pre-massaged tensors.
"""

import numpy as np
import ml_dtypes

HIDDEN = 512
HORIZON = 24
B_FULL = 1024
T_FULL = 256
N_CORES = 8
B_CORE = B_FULL // N_CORES  # 128
KC = HIDDEN // 128          # 4 chunks of the hidden dim
# batch-column split inside each core; widths must be EVEN: fp8 DoubleRow
# matmuls wedge the exec unit (NRT_EXEC_UNIT_UNRECOVERABLE) with an odd
# moving-operand width (empirically: n=43 wedges, 32/42/44 are fine)
GROUPS = (64, 64)

_COMPILED = {}


def build_kernel(T=T_FULL, use_bf16=True):
    import concourse.bass as bass
    import concourse.mybir as mybir
    import concourse.tile as tile
    from concourse.bass import ts

    dt = mybir.dt.bfloat16 if use_bf16 else mybir.dt.float32
    f32 = mybir.dt.float32
    f8 = mybir.dt.float8e4
    DR = mybir.MatmulPerfMode.DoubleRow

    nc = bass.Bass("TRN2", target_bir_lowering=False, debug=False,
                   num_devices=N_CORES)

    # ---- DRAM I/O (per-core shapes; host pre-massages layouts) ----
    # xw packs the x-outer weights and the input sequence in one tensor so
    # step 0 is gated by a single DMA: xw[:, m, :] for m<4 holds
    # [Wx_w ; Wx_b+Wh_b] chunk m (each [2, 128]), xw[:, 4+t, :] holds
    # [x[:, t] ; ones].
    xw_d = nc.dram_tensor("xw", [2, 4 + T, B_CORE], dt,
                          kind="ExternalInput").ap()
    # Wh in fp8 DoubleRow layout [128, 2(hi/lo), 2(q), 2(i), H]:
    # wh8[p, l, q, i, m] = W_l[m, (2q+i)*128 + p]  (l=0: hi, l=1: lo);
    # one tensor so the startup load is a single DMA (queue slots and
    # transfers serialize, so fewer DMAs land the weights sooner)
    wh8_d = nc.dram_tensor("wh8", [128, 2, 2, 2, HIDDEN], f8,
                           kind="ExternalInput").ap()
    # fcT arranged [128, KC, HORIZON]: fcT[p, k, n] = fc_w[n, k*128+p]
    fcT_d = nc.dram_tensor("fcT", [128, KC, HORIZON], dt, kind="ExternalInput").ap()
    # fc_b as column [HORIZON, 1] fp32
    fcb_d = nc.dram_tensor("fcb", [HORIZON, 1], f32, kind="ExternalInput").ap()
    # output [HORIZON, B_CORE] fp32 (host transposes/concats)
    out_d = nc.dram_tensor("out", [HORIZON, B_CORE], f32, kind="ExternalOutput").ap()

    ng = len(GROUPS)
    goff = [sum(GROUPS[:i]) for i in range(ng)]  # column offsets per group

    with tile.TileContext(nc) as tc:
        with (
            tc.tile_pool(name="consts", bufs=1) as cpool,
            tc.tile_pool(name="h", bufs=3) as hpool,
            tc.tile_pool(name="ps", bufs=2, space="PSUM") as pspool,
            tc.tile_pool(name="psfc", bufs=1, space="PSUM") as fcpool,
            tc.tile_pool(name="fin", bufs=1) as finpool,
        ):
            # ---- load constants into SBUF ----
            # Startup latency: each DMA costs a ~625 ns queue slot plus
            # ~900 ns of completion-sem propagation, and both the queue
            # slots and the transfers serialize.  Order the loads by when
            # the data is first needed: step 0 needs a head slice of x and
            # the (tiny) wxb; the fp8 weights are first read at step 1; the
            # x tail has until step T_HEAD (~16 us); fcT/fcb until the end.
            T_HEAD = 4 + min(16, T)
            xw_sb = cpool.tile([2, 4 + T, B_CORE], dt)
            nc.sync.dma_start(xw_sb[:, 0:T_HEAD], xw_d[:, 0:T_HEAD])
            # hi half first: step 1 runs hi-only (see below) so only this
            # transfer gates the pipeline rampup; the lo half lands during
            # step 1's compute
            wh8_sb = cpool.tile([128, 2, 2, 2, HIDDEN], f8)
            nc.sync.dma_start(wh8_sb[:, 0], wh8_d[:, 0])
            nc.sync.dma_start(wh8_sb[:, 1], wh8_d[:, 1])
            nc.sync.dma_start(xw_sb[:, T_HEAD:4 + T], xw_d[:, T_HEAD:4 + T])
            fcT_sb = cpool.tile([128, KC, HORIZON], dt)
            nc.sync.dma_start(fcT_sb[:], fcT_d[:])

            # h[g] tiles: [128, KC, n_g] (hidden-feature partition,
            # k-chunk x batch-col free)
            h = [None] * ng
            h_old = [None] * ng
            ps_fc = pspool.tile([HORIZON, B_CORE], f32, tag="psfc")
            out_sb = finpool.tile([HORIZON, B_CORE], f32)

            for t in range(T):
                for g in range(ng):
                    n = GROUPS[g]
                    o = goff[g]
                    ps = pspool.tile([128, KC * n], f32, tag=f"ps{g}")

                    # phase 0: x-outer + bias (K=2); first matmul into the
                    # bank carries start=True (marks whole bank pending-zero)
                    for m in range(KC):
                        nc.tensor.matmul(ps[:, ts(m, n)],
                                         xw_sb[0:2, m, :],
                                         xw_sb[0:2, 4 + t, o:o + n],
                                         start=(m == 0),
                                         stop=(t == 0 and m == KC - 1))
                    if t > 1:
                        # stale-lo: the lo-half matmuls read h[t-2] -- the
                        # W_lo*(h[t-1]-h[t-2]) error is ~1% of z per step
                        # (same order as the fp8 h quantization noise;
                        # measured end-to-end rel err 0.0144 vs 0.0120 for
                        # exact lo, gate 2e-2).  This takes 8 of the 16
                        # recurrent matmuls OFF the serial tanh chain, which
                        # is what makes the 2-group schedule (797 ns of ACT
                        # work + 944 ns chain) beat the 3-group one (982 ns
                        # of ACT work).
                        for q in range(2):
                            for m in range(KC):
                                nc.tensor.matmul(
                                    ps[:, ts(m, n)],
                                    wh8_sb[:, 1, q, :, ts(m, 128)],
                                    h_old[g][:, 2 * q:2 * q + 2, :],
                                    start=False, stop=False,
                                    perf_mode=DR)
                    if t > 0:
                        # hi-half matmuls on h[t-1]: the only matmuls on the
                        # serial chain, emitted last so the PSUM completes
                        # right when they finish
                        for q in range(2):
                            for m in range(KC):
                                nc.tensor.matmul(
                                    ps[:, ts(m, n)],
                                    wh8_sb[:, 0, q, :, ts(m, 128)],
                                    h[g][:, 2 * q:2 * q + 2, :],
                                    start=False,
                                    stop=(q == 1 and m == KC - 1),
                                    perf_mode=DR)
                    if t < T - 1:
                        h_new = hpool.tile([128, KC, n], f8, tag=f"h{g}")
                    else:
                        # final h in bf16: feeds the fc projection, where
                        # an fp8 h would add ~2e-2 output error
                        h_new = hpool.tile([128, KC, n], dt, tag=f"hf{g}")
                    nc.scalar.activation(h_new[:], ps[:],
                                         mybir.ActivationFunctionType.Tanh)
                    h_old[g] = h[g]
                    h[g] = h_new

                    if t == T - 1:
                        # final projection for this group as soon as its h
                        # lands: out[:, o:o+n] = sum_k fcT[k].T @ h_g[k]
                        # (fc_b is added on the host).  The g0+g1 piece is
                        # copied and stored while group 2 still finishes;
                        # the second store uses the other HWDGE queue so
                        # the two out-DMAs do not serialize.
                        for k in range(KC):
                            nc.tensor.matmul(ps_fc[:, o:o + n],
                                             fcT_sb[:, k, :],
                                             h[g][:, k, :],
                                             start=(g == 0 and k == 0),
                                             stop=(g == ng - 1 and
                                                   k == KC - 1))
                        if g == 1:
                            split = o + n
                            nc.scalar.activation(
                                out_sb[:, 0:split], ps_fc[:, 0:split],
                                mybir.ActivationFunctionType.Copy)
                            nc.sync.dma_start(out_d[:, 0:split],
                                              out_sb[:, 0:split])
                        elif g == 2:
                            split = o
                            nc.scalar.activation(
                                out_sb[:, split:], ps_fc[:, split:],
                                mybir.ActivationFunctionType.Copy)
                            nc.scalar.dma_start(out_d[:, split:],
                                                out_sb[:, split:])



    _strip_redundant_self_waits(nc)
    return nc


_SELF_SEM_PREFIX = {
    "InstActivation": "Activation",
    "InstMatmult": "PE",
    "InstLdweights": "PE",
    "InstTensorTensor": "DVE",
    "InstTensorScalarPtr": "DVE",
    "InstTensorCopy": "DVE",
}


def _strip_redundant_self_waits(nc):
    """Drop same-engine semaphore waits from instructions that carry more
    than one sync wait.

    Rationale: the HW engine instruction structs (MM/AC) hold only ONE
    sync-wait command; walrus refuses to codegen instructions with two.
    Tile emits a wait on the instruction's own engine sem for WAW/WAR on
    recycled tile-pool slots, but each engine executes its queue strictly
    in order, so ordering vs. its own earlier instructions is guaranteed
    without the wait.  Cross-engine waits are preserved; sem update counts
    are untouched (no other wait thresholds shift).
    """
    # Semaphore updated by the final DMA store of the "out" tensor; the
    # kernel-tail drain only genuinely needs this one (everything else is
    # transitively ordered: input DMAs -> compute -> final ACT -> out DMA).
    out_dma_sems = set()
    for b in nc.m.functions[0].blocks:
        for i in b.instructions:
            if type(i).__name__ != "InstDMACopy":
                continue
            names = [getattr(ap, "memref", "") for ap in i.outs]
            if "out" in names:
                si = i.sync_info
                if si:
                    out_dma_sems.update(u.ant_name for u in si.on_update)

    for b in nc.m.functions[0].blocks:
        for i in b.instructions:
            si = i.sync_info
            if si is None:
                continue
            ow = si.on_wait
            if len(ow) < 2:
                continue
            tname = type(i).__name__
            if tname == "InstDrain" and any(
                w.ant_name in out_dma_sems for w in ow
            ):
                si.on_wait = [w for w in ow if w.ant_name in out_dma_sems][:1]
                continue
            if tname == "InstDMACopy":
                # Keep the compute-engine wait (real data dependency);
                # drop stale cross-queue DMAHW waits (no data dependency:
                # all earlier DMAs here are input preloads this store
                # does not read, and same-ring descriptors are ordered
                # by the ring itself).
                kept = [w for w in ow if not w.ant_name.startswith("DMA")]
                if kept and len(kept) < len(ow):
                    si.on_wait = kept
                continue
            self_prefix = _SELF_SEM_PREFIX.get(tname)
            if self_prefix is None:
                continue
            kept = [w for w in ow if not w.ant_name.startswith(self_prefix)]
            if kept and len(kept) < len(ow):
                si.on_wait = kept


def _prep_inputs(x, Wx_w, Wx_b, Wh_w, Wh_b, fc_w, fc_b, T, use_bf16):
    """Host-side shard + layout massaging. Returns per-core input maps."""
    dt = ml_dtypes.bfloat16 if use_bf16 else np.float32
    f8 = ml_dtypes.float8_e4m3
    bias = (Wx_b + Wh_b).astype(np.float32)


    # Wh hi/lo fp8 split in DoubleRow layout [p, q, i, m]:
    # wh8*[p, q, i, m] = W*[m, (2q+i)*128 + p]
    W = Wh_w.astype(np.float32)
    W_hi = W.astype(f8).astype(np.float32)
    W_lo = W - W_hi

    def dr_layout(Wf):                                  # [512, 512] -> [128,2,2,512]
        # WT[k, m] = Wf[m, k]; reshape k = (q, i, p)
        return (Wf.T.reshape(2, 2, 128, HIDDEN)
                .transpose(2, 0, 1, 3).copy().astype(f8))

    wh8 = np.stack([dr_layout(W_hi), dr_layout(W_lo)], axis=1)  # [128,2,2,2,H]
    fcT = (fc_w.T.astype(np.float32)
           .reshape(KC, 128, HORIZON).transpose(1, 0, 2).copy().astype(dt))
    fcb = fc_b.astype(np.float32).reshape(HORIZON, 1).copy()

    in_maps = []
    for c in range(N_CORES):
        xs = x[c * B_CORE:(c + 1) * B_CORE, :T]                          # [128, T]
        x2T = np.empty((2, T, B_CORE), dtype=np.float32)
        x2T[0] = xs.T
        x2T[1] = 1.0
        in_maps.append({
            "x2T": x2T.astype(dt),
            "wxb": wxb,
            "wh8": wh8,
            "fcT": fcT,
            "fcb": fcb,
        })
    return in_maps


def kernel(x, Wx_w, Wx_b, Wh_w, Wh_b, fc_w, fc_b, _T=T_FULL, _bf16=True,
           _trace=False):
    from concourse.bass_utils import run_bass_kernel_spmd

    key = (_T, _bf16)
    if key not in _COMPILED:
        _COMPILED[key] = build_kernel(T=_T, use_bf16=_bf16)
    nc = _COMPILED[key]

    in_maps = _prep_inputs(x, Wx_w, Wx_b, Wh_w, Wh_b, fc_w, fc_b, _T, _bf16)
    res = run_bass_kernel_spmd(nc, in_maps, list(range(N_CORES)), trace=_trace)
    outs = [res.results[c]["out"] for c in range(N_CORES)]               # [24, 128] each
    full = np.concatenate(outs, axis=1).T.astype(np.float32).copy()      # [1024, 24]
    kernel._last_result = res
    return full
